# revision 11
# baseline (speedup 1.0000x reference)
# Trainium2 Bass kernel for LocationSensitiveAttention (Tacotron2-style).
#
# Full-input contract: kernel(**inputs) takes the complete unsharded inputs
# and returns (context, new_attn, new_cum) exactly like the jax reference.
# Internally the batch (256) is sharded 32-per-core across 8 NeuronCores
# (data parallel); the small parameters are replicated.
#
# Math notes (per batch element b):
#   cat      = stack([attn_weights, attn_weights_cum])            [2, T]
#   loc      = conv1d(cat, conv_w, k=31, same)                    [32, T]
#   p_loc    = W_loc @ loc                                        [128, T]
#            = WW^T.T @ im2col(cat_padded)   where WW^T[ck, a] = sum_f conv_w[f,c,k] W_loc[a,f]
#   p_query  = W_q @ query[b]                                     [128]
#   hidden   = tanh(p_loc + p_query + p_mem^T)                    [128, T]
#   energy   = v . hidden  (+ v_b, irrelevant: constant shift pre-softmax)
#   attn     = softmax(mask(energy))                              [T]
#   context  = attn @ memory[b]                                   [512]
#   cum_out  = cum_in + attn
#
# Layout: attention dim (128) on partitions, time on the free axis.
# processed_memory and query are pre-transposed on the host (pure layout).
# All large matmuls run in float32r (full-rate fp32 streaming mode, ~1.5e-4
# matmul rel err vs 4x-slower plain fp32); operands are rounded to f32r via
# DVE/ACT copies as the walrus verifier requires.

import numpy as np

B_TOTAL = 256
N_CORES = 8
B = B_TOTAL // N_CORES  # 32 per core
T = 1024
RNN = 1024
ENC = 512
ATT = 128
NF = 32
KS = 31
PAD = (KS - 1) // 2  # 15
CK = 2 * KS  # 62 contraction size for fused conv matmul
TPAD = T + 2 * PAD + 2  # 1056, slack so every im2col row read stays in range
TCH = T // 128  # 8 time chunks of 128

_CACHE = {}


def _build_bass(reps=1):
    # reps>1 repeats the whole computation (identical results) purely so
    # timing harnesses can difference out the per-execution launch overhead.
    import concourse.bacc as bacc
    import concourse.bass as bass
    import concourse.mybir as mybir
    import concourse.tile as tile

    f32 = mybir.dt.float32
    f32r = mybir.dt.float32r
    AF = mybir.ActivationFunctionType
    AX = mybir.AxisListType
    ALU = mybir.AluOpType

    nc = bacc.Bacc(
        "TRN2", target_bir_lowering=False, debug=False, num_devices=N_CORES
    )

    # ---- DRAM I/O -------------------------------------------------------
    mem_d = nc.dram_tensor("mem", [B, T, ENC], f32, kind="ExternalInput")
    pmemT_d = nc.dram_tensor("pmemT", [B, ATT, T], f32, kind="ExternalInput")
    catpad_d = nc.dram_tensor("catpad", [B, 2, TPAD], f32, kind="ExternalInput")
    qT_d = nc.dram_tensor("queryT", [RNN, B], f32, kind="ExternalInput")
    wqT_d = nc.dram_tensor("wqT", [RNN, ATT], f32, kind="ExternalInput")
    v32_d = nc.dram_tensor("v32", [ATT, B, B], f32, kind="ExternalInput")
    maskb_d = nc.dram_tensor("maskb", [B, T], f32, kind="ExternalInput")
    cum_d = nc.dram_tensor("cum", [B, T], f32, kind="ExternalInput")
    wlocT_d = nc.dram_tensor("wlocT", [NF, ATT], f32, kind="ExternalInput")
    convw_d = nc.dram_tensor("convw", [NF, CK], f32, kind="ExternalInput")

    ctx_d = nc.dram_tensor("ctx", [B, ENC], f32, kind="ExternalOutput")
    attn_d = nc.dram_tensor("attn_out", [B, T], f32, kind="ExternalOutput")
    cumo_d = nc.dram_tensor("cum_out", [B, T], f32, kind="ExternalOutput")

    id32_d = nc.inline_tensor(np.eye(32, dtype=np.float32), name="id32")

    with tile.TileContext(nc) as tc:
      for _rep in range(reps):
        with (
            tc.tile_pool(name="const", bufs=1) as cst,
            tc.tile_pool(name="psmisc", bufs=1, space="PSUM") as psm,
            tc.tile_pool(name="pe32", bufs=1, space="PSUM") as pe32p,
        ):
            # ---- constant / parameter loads -----------------------------
            wq_sb = cst.tile([128, RNN // 128, ATT], f32)
            nc.gpsimd.dma_start(
                wq_sb[:], wqT_d.ap().rearrange("(c p) a -> p c a", p=128)
            )
            qT_sb = cst.tile([128, RNN // 128, B], f32)
            nc.gpsimd.dma_start(
                qT_sb[:], qT_d.ap().rearrange("(c p) b -> p c b", p=128)
            )
            v32_sb = cst.tile([ATT, B, B], f32)
            nc.gpsimd.dma_start(v32_sb[:], v32_d.ap())
            maskb_sb = cst.tile([B, T], f32)
            nc.gpsimd.dma_start(maskb_sb[:], maskb_d.ap())
            cum_sb = cst.tile([B, T], f32)
            nc.gpsimd.dma_start(cum_sb[:], cum_d.ap())
            wloc_sb = cst.tile([NF, ATT], f32)
            nc.gpsimd.dma_start(wloc_sb[:], wlocT_d.ap())
            convw_sb = cst.tile([NF, CK], f32)
            nc.gpsimd.dma_start(convw_sb[:], convw_d.ap())
            id32_sb = cst.tile([32, 32], f32)
            nc.gpsimd.dma_start(id32_sb[:], id32_d.ap())

            # f32r-rounded copies of the matmul operands that come from DMA
            v32_r = cst.tile([ATT, B, B], f32r)
            nc.vector.tensor_copy(v32_r[:], v32_sb[:])

            # ---- p_query^T = W_q @ query^T  -> [ATT, B] -----------------
            ps_pq = psm.tile([ATT, B], f32)
            for c in range(RNN // 128):
                nc.tensor.matmul(
                    ps_pq[:],
                    wq_sb[:, c, :],
                    qT_sb[:, c, :],
                    start=(c == 0),
                    stop=(c == RNN // 128 - 1),
                )
            pq_sb = cst.tile([ATT, B], f32)
            nc.vector.tensor_copy(pq_sb[:], ps_pq[:])

            # ---- WW^T[ck, a] = sum_f convw[f, ck] * W_loc^T[f, a] -------
            ps_ww = psm.tile([CK, ATT], f32)
            nc.tensor.matmul(ps_ww[:], convw_sb[:], wloc_sb[:], start=True, stop=True)
            wwt_r = cst.tile([CK, ATT], f32r)
            nc.vector.tensor_copy(wwt_r[:], ps_ww[:])

            # ---- phase 1: energies for each batch row -------------------
            pe32_t = pe32p.tile([B, T], f32)  # accumulated energies, all rows

            with (
                tc.tile_pool(name="work", bufs=3) as wk,
                tc.tile_pool(name="workr", bufs=2) as wkr,
                tc.tile_pool(name="ploc", bufs=2, space="PSUM") as plocp,
            ):
                hid_prev = None
                for b in range(B + 1):
                    if b < B:
                        im = wk.tile([CK, T], f32, tag="im")
                        # overlapping-window (im2col) read of padded cat:
                        # partition (c,k) <- catpad[b, c, k : k+T]
                        nc.gpsimd.dma_start(
                            im[:],
                            bass.AP(
                                catpad_d, b * 2 * TPAD, [[TPAD, 2], [1, KS], [1, T]]
                            ),
                        )
                        imr = wkr.tile([CK, T], f32r, tag="imr")
                        nc.vector.tensor_copy(imr[:], im[:])
                        pm = wk.tile([ATT, T], f32, tag="pm")
                        nc.scalar.dma_start(pm[:], pmemT_d.ap()[b])

                        pl = plocp.tile([ATT, T], f32, tag="pl")
                        for h in range(2):
                            sl = slice(h * 512, (h + 1) * 512)
                            nc.tensor.matmul(
                                pl[:, sl], wwt_r[:], imr[:, sl], start=True, stop=True
                            )
                        # hidden_pre = (p_loc + p_query) + p_mem on DVE, then tanh
                        hpre = wkr.tile([ATT, T], f32, tag="hpre")
                        nc.vector.scalar_tensor_tensor(
                            hpre[:], pl[:], pq_sb[:, b : b + 1], pm[:],
                            op0=ALU.add, op1=ALU.add,
                        )
                        hid = wkr.tile([ATT, T], f32r, tag="hid")
                        nc.scalar.activation(hid[:], hpre[:], AF.Tanh)
                    if b >= 1:
                        j = b - 1
                        for h in range(2):
                            sl = slice(h * 512, (h + 1) * 512)
                            nc.tensor.matmul(
                                pe32_t[:, sl],
                                v32_r[:, j, :],
                                hid_prev[:, sl],
                                start=(j == 0),
                                stop=(j == B - 1),
                            )
                    hid_prev = hid

            # ---- softmax over time, all 32 rows at once -----------------
            with tc.tile_pool(name="smx", bufs=1) as smx:
                em = smx.tile([B, T], f32)
                nc.vector.tensor_add(em[:], pe32_t[:], maskb_sb[:])
                nmax = smx.tile([B, 1], f32)
                nc.vector.tensor_reduce(
                    nmax[:], em[:], axis=AX.X, op=ALU.max, negate=True
                )
                p_sb = smx.tile([B, T], f32)
                rs = smx.tile([B, 1], f32)
                nc.scalar.activation(
                    p_sb[:], em[:], AF.Exp, bias=nmax[:], accum_out=rs[:]
                )
                ri = smx.tile([B, 1], f32)
                nc.vector.reciprocal(ri[:], rs[:])
                attn_sb = smx.tile([B, T], f32)
                nc.vector.tensor_scalar_mul(attn_sb[:], p_sb[:], ri[:])
                cumo_sb = smx.tile([B, T], f32)
                nc.vector.tensor_add(cumo_sb[:], attn_sb[:], cum_sb[:])
                nc.scalar.dma_start(attn_d.ap(), attn_sb[:])
                nc.scalar.dma_start(cumo_d.ap(), cumo_sb[:])

                # ---- transpose attn into [t, b] columns -----------------
                with (
                    tc.tile_pool(name="atp", bufs=TCH) as atp,
                    tc.tile_pool(name="ptp", bufs=2, space="PSUM") as ptp,
                ):
                    attnT = []
                    for c in range(TCH):
                        pt = ptp.tile([128, B], f32, tag="pt")
                        nc.tensor.transpose(
                            pt[:], attn_sb[:, c * 128 : (c + 1) * 128], id32_sb[:]
                        )
                        at = atp.tile([128, B], f32r, tag="at")
                        nc.vector.tensor_copy(at[:], pt[:])
                        attnT.append(at)

                    # ---- phase 3: context = attn @ memory ---------------
                    with (
                        tc.tile_pool(name="memp", bufs=7) as memp,
                        tc.tile_pool(name="memr", bufs=3) as memrp,
                        tc.tile_pool(name="pctx", bufs=2, space="PSUM") as pcp,
                        tc.tile_pool(name="cout", bufs=2) as cop,
                    ):
                        for b in range(B):
                            mt = memp.tile([128, TCH, ENC], f32, tag="mt")
                            # alternate HWDGE rings so consecutive 2 MiB loads
                            # overlap their completion latencies
                            eng = nc.sync if b % 2 == 0 else nc.scalar
                            eng.dma_start(
                                mt[:],
                                mem_d.ap()[b].rearrange("(c p) e -> p c e", p=128),
                            )
                            mr = memrp.tile([128, TCH, ENC], f32r, tag="mr")
                            # alternate the rounding pass between DVE and ACT
                            if b % 2 == 0:
                                nc.vector.tensor_copy(mr[:], mt[:])
                            else:
                                nc.scalar.copy(mr[:], mt[:])
                            pc = pcp.tile([1, ENC], f32, tag="pc")
                            for c in range(TCH):
                                nc.tensor.matmul(
                                    pc[:],
                                    attnT[c][:, b : b + 1],
                                    mr[:, c, :],
                                    start=(c == 0),
                                    stop=(c == TCH - 1),
                                )
                            co = cop.tile([1, ENC], f32, tag="co")
                            nc.vector.tensor_copy(co[:], pc[:])
                            nc.gpsimd.dma_start(ctx_d.ap()[b], co[:])

    nc.compile()
    return nc


def _get_nc(reps=1):
    key = f"nc{reps}"
    if key not in _CACHE:
        _CACHE[key] = _build_bass(reps)
    return _CACHE[key]


def _prepare_in_maps(inputs):
    query = np.asarray(inputs["query"], dtype=np.float32)
    memory = np.asarray(inputs["memory"], dtype=np.float32)
    pmem = np.asarray(inputs["processed_memory"], dtype=np.float32)
    aw = np.asarray(inputs["attn_weights"], dtype=np.float32)
    awc = np.asarray(inputs["attn_weights_cum"], dtype=np.float32)
    lens = np.asarray(inputs["memory_lengths"]).astype(np.int64)
    conv_w = np.asarray(inputs["conv_w"], dtype=np.float32)
    W_loc = np.asarray(inputs["W_loc"], dtype=np.float32)
    W_q = np.asarray(inputs["W_q"], dtype=np.float32)
    v_w = np.asarray(inputs["v_w"], dtype=np.float32)
    # v_b shifts every energy by the same constant -> softmax-invariant; ignore.

    # Shared (replicated) params, pure layout transforms on the host.
    wqT = np.ascontiguousarray(W_q.T)  # [RNN, ATT]
    wlocT = np.ascontiguousarray(W_loc.T)  # [NF, ATT]
    convw = np.ascontiguousarray(conv_w.reshape(NF, CK))  # [NF, (c k)]
    v32 = np.zeros((ATT, B, B), dtype=np.float32)
    v32[:, np.arange(B), np.arange(B)] = v_w[0][:, None]

    catpad = np.zeros((B_TOTAL, 2, TPAD), dtype=np.float32)
    catpad[:, 0, PAD : PAD + T] = aw
    catpad[:, 1, PAD : PAD + T] = awc

    maskb = np.where(
        np.arange(T)[None, :] >= lens[:, None], np.float32(-1e30), np.float32(0)
    ).astype(np.float32)

    pmemT = np.ascontiguousarray(pmem.transpose(0, 2, 1))  # [B_TOTAL, ATT, T]
    queryT = np.ascontiguousarray(query.T)  # [RNN, B_TOTAL]

    in_maps = []
    for c in range(N_CORES):
        s = slice(c * B, (c + 1) * B)
        in_maps.append(
            {
                "mem": np.ascontiguousarray(memory[s]),
                "pmemT": np.ascontiguousarray(pmemT[s]),
                "catpad": np.ascontiguousarray(catpad[s]),
                "queryT": np.ascontiguousarray(queryT[:, s]),
                "wqT": wqT,
                "v32": v32,
                "maskb": np.ascontiguousarray(maskb[s]),
                "cum": np.ascontiguousarray(awc[s]),
                "wlocT": wlocT,
                "convw": convw,
            }
        )
    return in_maps


def _run(inputs, trace=False):
    from concourse import bass_utils

    nc = _get_nc()
    in_maps = _prepare_in_maps(inputs)
    res = bass_utils.run_bass_kernel_spmd(
        nc, in_maps, core_ids=list(range(N_CORES)), trace=trace
    )
    ctx = np.concatenate([r["ctx"] for r in res.results], axis=0)
    attn = np.concatenate([r["attn_out"] for r in res.results], axis=0)
    cum = np.concatenate([r["cum_out"] for r in res.results], axis=0)
    return (ctx, attn, cum), res


def kernel(**inputs):
    out, _ = _run(inputs, trace=False)
    return out


# revision 12
# speedup vs baseline: 6.1747x; 6.1747x over previous
# Trainium2 Bass kernel for LocationSensitiveAttention (Tacotron2-style).
#
# Full-input contract: kernel(**inputs) takes the complete unsharded inputs
# and returns (context, new_attn, new_cum) exactly like the jax reference.
# Internally the batch (256) is sharded 32-per-core across 8 NeuronCores
# (data parallel); the small parameters are replicated.
#
# Math notes (per batch element b):
#   cat      = stack([attn_weights, attn_weights_cum])            [2, T]
#   loc      = conv1d(cat, conv_w, k=31, same)                    [32, T]
#   p_loc    = W_loc @ loc                                        [128, T]
#            = WW^T.T @ im2col(cat_padded)   where WW^T[ck, a] = sum_f conv_w[f,c,k] W_loc[a,f]
#   p_query  = W_q @ query[b]                                     [128]
#   hidden   = tanh(p_loc + p_query + p_mem^T)                    [128, T]
#   energy   = v . hidden  (+ v_b, irrelevant: constant shift pre-softmax)
#   attn     = softmax(mask(energy))                              [T]
#   context  = attn @ memory[b]                                   [512]
#   cum_out  = cum_in + attn
#
# Layout: attention dim (128) on partitions, time on the free axis.
# processed_memory and query are pre-transposed on the host (pure layout).
# All large matmuls run in float32r (full-rate fp32 streaming mode, ~1.5e-4
# matmul rel err vs 4x-slower plain fp32); operands are rounded to f32r via
# DVE/ACT copies as the walrus verifier requires.

import numpy as np

B_TOTAL = 256
N_CORES = 8
B = B_TOTAL // N_CORES  # 32 per core
T = 1024
RNN = 1024
ENC = 512
ATT = 128
NF = 32
KS = 31
PAD = (KS - 1) // 2  # 15
CK = 2 * KS  # 62 contraction size for fused conv matmul
TPAD = T + 2 * PAD + 2  # 1056, slack so every im2col row read stays in range
TCH = T // 128  # 8 time chunks of 128

_CACHE = {}


def _build_bass(reps=1):
    # reps>1 repeats the whole computation (identical results) purely so
    # timing harnesses can difference out the per-execution launch overhead.
    import concourse.bacc as bacc
    import concourse.bass as bass
    import concourse.mybir as mybir
    import concourse.tile as tile

    f32 = mybir.dt.float32
    f32r = mybir.dt.float32r
    AF = mybir.ActivationFunctionType
    AX = mybir.AxisListType
    ALU = mybir.AluOpType

    nc = bacc.Bacc(
        "TRN2", target_bir_lowering=False, debug=False, num_devices=N_CORES
    )

    # ---- DRAM I/O -------------------------------------------------------
    mem_d = nc.dram_tensor("mem", [B, T, ENC], f32, kind="ExternalInput")
    pmemT_d = nc.dram_tensor("pmemT", [B, ATT, T], f32, kind="ExternalInput")
    catpad_d = nc.dram_tensor("catpad", [B, 2, TPAD], f32, kind="ExternalInput")
    qT_d = nc.dram_tensor("queryT", [RNN, B], f32, kind="ExternalInput")
    wqT_d = nc.dram_tensor("wqT", [RNN, ATT], f32, kind="ExternalInput")
    v32_d = nc.dram_tensor("v32", [ATT, B, B], f32, kind="ExternalInput")
    maskb_d = nc.dram_tensor("maskb", [B, T], f32, kind="ExternalInput")
    cum_d = nc.dram_tensor("cum", [B, T], f32, kind="ExternalInput")
    wlocT_d = nc.dram_tensor("wlocT", [NF, ATT], f32, kind="ExternalInput")
    convw_d = nc.dram_tensor("convw", [NF, CK], f32, kind="ExternalInput")

    ctx_d = nc.dram_tensor("ctx", [B, ENC], f32, kind="ExternalOutput")
    attn_d = nc.dram_tensor("attn_out", [B, T], f32, kind="ExternalOutput")
    cumo_d = nc.dram_tensor("cum_out", [B, T], f32, kind="ExternalOutput")

    id32_d = nc.inline_tensor(np.eye(32, dtype=np.float32), name="id32")

    with tile.TileContext(nc) as tc:
      for _rep in range(reps):
        with (
            tc.tile_pool(name="const", bufs=1) as cst,
            tc.tile_pool(name="psmisc", bufs=1, space="PSUM") as psm,
            tc.tile_pool(name="pe32", bufs=1, space="PSUM") as pe32p,
        ):
            # ---- constant / parameter loads -----------------------------
            wq_sb = cst.tile([128, RNN // 128, ATT], f32)
            nc.gpsimd.dma_start(
                wq_sb[:], wqT_d.ap().rearrange("(c p) a -> p c a", p=128)
            )
            qT_sb = cst.tile([128, RNN // 128, B], f32)
            nc.gpsimd.dma_start(
                qT_sb[:], qT_d.ap().rearrange("(c p) b -> p c b", p=128)
            )
            v32_sb = cst.tile([ATT, B, B], f32)
            nc.gpsimd.dma_start(v32_sb[:], v32_d.ap())
            maskb_sb = cst.tile([B, T], f32)
            nc.gpsimd.dma_start(maskb_sb[:], maskb_d.ap())
            cum_sb = cst.tile([B, T], f32)
            nc.gpsimd.dma_start(cum_sb[:], cum_d.ap())
            wloc_sb = cst.tile([NF, ATT], f32)
            nc.gpsimd.dma_start(wloc_sb[:], wlocT_d.ap())
            convw_sb = cst.tile([NF, CK], f32)
            nc.gpsimd.dma_start(convw_sb[:], convw_d.ap())
            id32_sb = cst.tile([32, 32], f32)
            nc.gpsimd.dma_start(id32_sb[:], id32_d.ap())

            # f32r-rounded copies of the matmul operands that come from DMA
            v32_r = cst.tile([ATT, B, B], f32r)
            nc.vector.tensor_copy(v32_r[:], v32_sb[:])

            # ---- p_query^T = W_q @ query^T  -> [ATT, B] -----------------
            ps_pq = psm.tile([ATT, B], f32)
            for c in range(RNN // 128):
                nc.tensor.matmul(
                    ps_pq[:],
                    wq_sb[:, c, :],
                    qT_sb[:, c, :],
                    start=(c == 0),
                    stop=(c == RNN // 128 - 1),
                )
            pq_sb = cst.tile([ATT, B], f32)
            nc.vector.tensor_copy(pq_sb[:], ps_pq[:])

            # ---- WW^T[ck, a] = sum_f convw[f, ck] * W_loc^T[f, a] -------
            ps_ww = psm.tile([CK, ATT], f32)
            nc.tensor.matmul(ps_ww[:], convw_sb[:], wloc_sb[:], start=True, stop=True)
            wwt_r = cst.tile([CK, ATT], f32r)
            nc.vector.tensor_copy(wwt_r[:], ps_ww[:])

            # ---- phase 1: energies for each batch row -------------------
            pe32_t = pe32p.tile([B, T], f32)  # accumulated energies, all rows

            with (
                tc.tile_pool(name="work", bufs=3) as wk,
                tc.tile_pool(name="workr", bufs=2) as wkr,
                tc.tile_pool(name="ploc", bufs=2, space="PSUM") as plocp,
            ):
                hid_prev = None
                for b in range(B + 1):
                    if b < B:
                        cat = wk.tile([2, TPAD], f32, tag="cat")
                        nc.gpsimd.dma_start(cat[:], catpad_d.ap()[b])
                        im = wk.tile([CK, T], f32, tag="im")
                        # overlapping-window (im2col) expansion on-chip:
                        # partition (c,k) <- cat[c, k : k+T], SBUF->SBUF
                        csrc = cat[:]
                        nc.gpsimd.dma_start(
                            im[:],
                            bass.AP(
                                csrc.tensor,
                                csrc.offset,
                                [[csrc.ap[0][0], 2], [1, KS], [1, T]],
                            ),
                        )
                        imr = wkr.tile([CK, T], f32r, tag="imr")
                        nc.vector.tensor_copy(imr[:], im[:])
                        pm = wk.tile([ATT, T], f32, tag="pm")
                        nc.scalar.dma_start(pm[:], pmemT_d.ap()[b])

                        pl = plocp.tile([ATT, T], f32, tag="pl")
                        for h in range(2):
                            sl = slice(h * 512, (h + 1) * 512)
                            nc.tensor.matmul(
                                pl[:, sl], wwt_r[:], imr[:, sl], start=True, stop=True
                            )
                        # hidden_pre = (p_loc + p_query) + p_mem on DVE, then tanh
                        hpre = wkr.tile([ATT, T], f32, tag="hpre")
                        nc.vector.scalar_tensor_tensor(
                            hpre[:], pl[:], pq_sb[:, b : b + 1], pm[:],
                            op0=ALU.add, op1=ALU.add,
                        )
                        hid = wkr.tile([ATT, T], f32r, tag="hid")
                        nc.scalar.activation(hid[:], hpre[:], AF.Tanh)
                    if b >= 1:
                        j = b - 1
                        for h in range(2):
                            sl = slice(h * 512, (h + 1) * 512)
                            nc.tensor.matmul(
                                pe32_t[:, sl],
                                v32_r[:, j, :],
                                hid_prev[:, sl],
                                start=(j == 0),
                                stop=(j == B - 1),
                            )
                    hid_prev = hid

            # ---- softmax over time, all 32 rows at once -----------------
            with tc.tile_pool(name="smx", bufs=1) as smx:
                em = smx.tile([B, T], f32)
                nc.vector.tensor_add(em[:], pe32_t[:], maskb_sb[:])
                nmax = smx.tile([B, 1], f32)
                nc.vector.tensor_reduce(
                    nmax[:], em[:], axis=AX.X, op=ALU.max, negate=True
                )
                p_sb = smx.tile([B, T], f32)
                rs = smx.tile([B, 1], f32)
                nc.scalar.activation(
                    p_sb[:], em[:], AF.Exp, bias=nmax[:], accum_out=rs[:]
                )
                ri = smx.tile([B, 1], f32)
                nc.vector.reciprocal(ri[:], rs[:])
                attn_sb = smx.tile([B, T], f32)
                nc.vector.tensor_scalar_mul(attn_sb[:], p_sb[:], ri[:])
                cumo_sb = smx.tile([B, T], f32)
                nc.vector.tensor_add(cumo_sb[:], attn_sb[:], cum_sb[:])
                nc.scalar.dma_start(attn_d.ap(), attn_sb[:])
                nc.scalar.dma_start(cumo_d.ap(), cumo_sb[:])

                # ---- transpose attn into [t, b] columns -----------------
                with (
                    tc.tile_pool(name="atp", bufs=TCH) as atp,
                    tc.tile_pool(name="ptp", bufs=2, space="PSUM") as ptp,
                ):
                    attnT = []
                    for c in range(TCH):
                        pt = ptp.tile([128, B], f32, tag="pt")
                        nc.tensor.transpose(
                            pt[:], attn_sb[:, c * 128 : (c + 1) * 128], id32_sb[:]
                        )
                        at = atp.tile([128, B], f32r, tag="at")
                        nc.vector.tensor_copy(at[:], pt[:])
                        attnT.append(at)

                    # ---- phase 3: context = attn @ memory ---------------
                    with (
                        tc.tile_pool(name="memp", bufs=7) as memp,
                        tc.tile_pool(name="memr", bufs=3) as memrp,
                        tc.tile_pool(name="pctx", bufs=2, space="PSUM") as pcp,
                        tc.tile_pool(name="cout", bufs=2) as cop,
                    ):
                        for b in range(B):
                            mt = memp.tile([128, TCH, ENC], f32, tag="mt")
                            # alternate HWDGE rings so consecutive 2 MiB loads
                            # overlap their completion latencies
                            eng = nc.sync if b % 2 == 0 else nc.scalar
                            eng.dma_start(
                                mt[:],
                                mem_d.ap()[b].rearrange("(c p) e -> p c e", p=128),
                            )
                            mr = memrp.tile([128, TCH, ENC], f32r, tag="mr")
                            # alternate the rounding pass between DVE and ACT
                            if b % 2 == 0:
                                nc.vector.tensor_copy(mr[:], mt[:])
                            else:
                                nc.scalar.copy(mr[:], mt[:])
                            pc = pcp.tile([1, ENC], f32, tag="pc")
                            for c in range(TCH):
                                nc.tensor.matmul(
                                    pc[:],
                                    attnT[c][:, b : b + 1],
                                    mr[:, c, :],
                                    start=(c == 0),
                                    stop=(c == TCH - 1),
                                )
                            co = cop.tile([1, ENC], f32, tag="co")
                            nc.vector.tensor_copy(co[:], pc[:])
                            nc.gpsimd.dma_start(ctx_d.ap()[b], co[:])

    nc.compile()
    return nc


def _get_nc(reps=1):
    key = f"nc{reps}"
    if key not in _CACHE:
        _CACHE[key] = _build_bass(reps)
    return _CACHE[key]


def _prepare_in_maps(inputs):
    query = np.asarray(inputs["query"], dtype=np.float32)
    memory = np.asarray(inputs["memory"], dtype=np.float32)
    pmem = np.asarray(inputs["processed_memory"], dtype=np.float32)
    aw = np.asarray(inputs["attn_weights"], dtype=np.float32)
    awc = np.asarray(inputs["attn_weights_cum"], dtype=np.float32)
    lens = np.asarray(inputs["memory_lengths"]).astype(np.int64)
    conv_w = np.asarray(inputs["conv_w"], dtype=np.float32)
    W_loc = np.asarray(inputs["W_loc"], dtype=np.float32)
    W_q = np.asarray(inputs["W_q"], dtype=np.float32)
    v_w = np.asarray(inputs["v_w"], dtype=np.float32)
    # v_b shifts every energy by the same constant -> softmax-invariant; ignore.

    # Shared (replicated) params, pure layout transforms on the host.
    wqT = np.ascontiguousarray(W_q.T)  # [RNN, ATT]
    wlocT = np.ascontiguousarray(W_loc.T)  # [NF, ATT]
    convw = np.ascontiguousarray(conv_w.reshape(NF, CK))  # [NF, (c k)]
    v32 = np.zeros((ATT, B, B), dtype=np.float32)
    v32[:, np.arange(B), np.arange(B)] = v_w[0][:, None]

    catpad = np.zeros((B_TOTAL, 2, TPAD), dtype=np.float32)
    catpad[:, 0, PAD : PAD + T] = aw
    catpad[:, 1, PAD : PAD + T] = awc

    maskb = np.where(
        np.arange(T)[None, :] >= lens[:, None], np.float32(-1e30), np.float32(0)
    ).astype(np.float32)

    pmemT = np.ascontiguousarray(pmem.transpose(0, 2, 1))  # [B_TOTAL, ATT, T]
    queryT = np.ascontiguousarray(query.T)  # [RNN, B_TOTAL]

    in_maps = []
    for c in range(N_CORES):
        s = slice(c * B, (c + 1) * B)
        in_maps.append(
            {
                "mem": np.ascontiguousarray(memory[s]),
                "pmemT": np.ascontiguousarray(pmemT[s]),
                "catpad": np.ascontiguousarray(catpad[s]),
                "queryT": np.ascontiguousarray(queryT[:, s]),
                "wqT": wqT,
                "v32": v32,
                "maskb": np.ascontiguousarray(maskb[s]),
                "cum": np.ascontiguousarray(awc[s]),
                "wlocT": wlocT,
                "convw": convw,
            }
        )
    return in_maps


def _run(inputs, trace=False):
    from concourse import bass_utils

    nc = _get_nc()
    in_maps = _prepare_in_maps(inputs)
    res = bass_utils.run_bass_kernel_spmd(
        nc, in_maps, core_ids=list(range(N_CORES)), trace=trace
    )
    ctx = np.concatenate([r["ctx"] for r in res.results], axis=0)
    attn = np.concatenate([r["attn_out"] for r in res.results], axis=0)
    cum = np.concatenate([r["cum_out"] for r in res.results], axis=0)
    return (ctx, attn, cum), res


def kernel(**inputs):
    out, _ = _run(inputs, trace=False)
    return out
